# revision 2
# baseline (speedup 1.0000x reference)
"""Distributed multi-head attention kernel for one TRN2 chip (8 NeuronCores).

Problem: nn_Attention_13048110645268
  x [2, 2048, 1024] f32 ->  attention(16 heads, d=64) -> out [2, 2048, 1024] f32

Sharding (Megatron-style batch x head-group):
  core c in [0,8): batch b = c//4, head group g = c%4 (heads 4g..4g+3).

v3 design: ScalarE-exp-clocked software pipeline with PE tile concurrency.
  - Heads processed as two pairs A=(0,1), B=(2,3); qk features of the even
    head on partitions 0-63, odd head on 64-127.
  - scores pair: two 64x128x512 MMs, lhsT base partitions 0/64 -> row-tiled,
    run concurrently on the PE.
  - AV pair: two 128x64x512 MMs into psum halves 0-63/64-127 -> col-tiled,
    concurrent.
  - exp: one [128,1024] ScalarE activation per pair per key tile; ScalarE
    is the pipeline clock (~2.5us/iter at the P0 clock).
  - rowsum: one 4-way-concurrent 128x1x512 quad per iteration accumulating
    into a dedicated PSUM bank (freed early by the boundary reciprocal).
  - PSUM banks: scores 2x[128,1024] (4) + AV accums 2x[128,512] (2) +
    rowsum (1) + filler scratch (1) = 8.
  - qk/v/proj run as deadline-scheduled fillers inside the exp shadow,
    with qk/proj groups split into 4-kd chunks across the two halves of
    one iteration.
  - 5 query blocks (512,512,512,256,256): the two final 256-query
    blocks let the second-to-last AllGather hide under attention and
    leave only a small AG exposed in the tail.
"""

import os
import sys

import numpy as np

sys.path.insert(0, "/opt/trn_rl_repo")

import ml_dtypes  # noqa: E402

import concourse.bass as bass  # noqa: E402
import concourse.mybir as mybir  # noqa: E402
import concourse.tile as tile  # noqa: E402
from concourse import bacc  # noqa: E402
from concourse.bass_utils import run_bass_kernel_spmd  # noqa: E402

BF16 = mybir.dt.bfloat16
F32 = mybir.dt.float32
NBF16 = ml_dtypes.bfloat16

B, S, D = 2, 2048, 1024
H, HD = 16, 64
NCORES = 8
GROUPS = [[0, 1, 2, 3], [4, 5, 6, 7]]
HL = 4          # heads per core
DL = HL * HD    # 256 feature dims per core
P = 128
KT = S // P     # 16 key tiles
QB = 4          # query blocks
QW = S // QB    # 512 queries per block
KD = D // P     # 8 contraction tiles over model dim
SCALE = HD ** -0.5
# (global query start, width) per processing block; last 512 split in two so
# the second-to-last AllGather hides under the final block's attention.
BLOCKS = [(0, 512), (512, 512), (1024, 512), (1536, 256), (1792, 256)]
NB = len(BLOCKS)

_CACHE = {}


def _restripe(w):
    """[KD*128, C] -> [128, KD*C] with row p holding all kd-subtiles."""
    kd = w.shape[0] // P
    return np.ascontiguousarray(
        w.reshape(kd, P, w.shape[1]).transpose(1, 0, 2).reshape(P, -1))


def _emit(nc: bass.Bass, tc: tile.TileContext, xT, wqk, wv, wp, bqk, beff, yT):
    exp_fn = mybir.ActivationFunctionType.Exp

    with (
        tc.tile_pool(name="main", bufs=1) as mp,
        tc.tile_pool(name="ep", bufs=4) as ep,
        tc.tile_pool(name="gp", bufs=2) as gp,
        tc.tile_pool(name="yp", bufs=2) as yp,
        tc.tile_pool(name="rp", bufs=4) as rp,
        tc.tile_pool(name="ps_sc", bufs=2, space="PSUM") as ps_sc,
        tc.tile_pool(name="ps_av", bufs=2, space="PSUM") as ps_av,
        tc.tile_pool(name="ps_rs", bufs=1, space="PSUM") as ps_rs_pool,
        tc.tile_pool(name="ps_mm", bufs=1, space="PSUM") as ps_mm,
        tc.tile_pool(name="dram", bufs=2, space="DRAM") as dp,
    ):
        # ---------------- input DMA (ordered by first use) ----------------
        xT_sb = mp.tile([P, QB, KD, 512], BF16)   # x^T [d-part, n, d-tile, tok]
        nc.sync.dma_start(xT_sb[:, 0],
                          xT[0, :, :].rearrange("p (kd u) -> p kd u", kd=KD))
        wqk_sb = mp.tile([P, KD, 2 * DL], BF16)
        nc.sync.dma_start(wqk_sb[:],
                          wqk[:, :].rearrange("p (kd c) -> p kd c", kd=KD))
        bqk_sb = mp.tile([P, 4], F32)
        nc.sync.dma_start(bqk_sb[:], bqk[:, :])
        wv_sb = mp.tile([P, KD, DL], BF16)
        nc.sync.dma_start(wv_sb[:],
                          wv[:, :].rearrange("p (kd c) -> p kd c", kd=KD))
        for n in range(1, QB):
            nc.sync.dma_start(xT_sb[:, n],
                              xT[n, :, :].rearrange("p (kd u) -> p kd u", kd=KD))
        wp_sb = mp.tile([P, KD, DL], BF16)
        nc.sync.dma_start(wp_sb[:],
                          wp[:, :].rearrange("p (kd c) -> p kd c", kd=KD))
        beff_sb = mp.tile([P, 2], F32)
        nc.sync.dma_start(beff_sb[:], beff[:, :])
        ones_sb = mp.tile([P, 64], BF16)
        nc.vector.memset(ones_sb[:], 1.0)
        warm_cc_in = dp.tile([8, 64], BF16, name="warm_cc_in")
        nc.sync.dma_start(warm_cc_in[:, :], xT[0, 0:8, 0:64])
        warm_cc_out = dp.tile([32, 64], BF16, name="warm_cc_out")
        nc.gpsimd.collective_compute(
            "AllGather",
            mybir.AluOpType.bypass,
            replica_groups=GROUPS,
            ins=[warm_cc_in[:, :].opt()],
            outs=[warm_cc_out[:, :].opt()],
        )

        # ---------------- persistent SBUF ----------------
        qkT_sb = mp.tile([P, 4, S], BF16)     # ct 0,1 = q pair A/B; 2,3 = k
        v_sb = mp.tile([P, KT, DL], BF16)
        outT_sb = mp.tile([P, QB, 2, 512], BF16)
        g_tiles = [None] * NB

        # PE warm-up: dummy matmuls with no input deps run while input DMAs
        # land, lifting the HAM clock gate before real work.
        warm_sb = mp.tile([P, 512], BF16)
        nc.vector.memset(warm_sb[:], 1.0)
        for w in range(10):
            ps_warm = ps_mm.tile([P, 512], F32, tag="mm", name="ps_mm")
            nc.tensor.matmul(ps_warm[:], lhsT=warm_sb[:, 0:P], rhs=warm_sb[:],
                             start=True, stop=True)

        # ---------------- building blocks ----------------
        _grp = [None]   # psum tile held across the chunks of one mm group

        def _grp_tile(lo):
            if lo == 0:
                _grp[0] = ps_mm.tile([P, 512], F32, tag="mm", name="ps_mm")
            return _grp[0]

        def emit_qk(n, ct, lo=0, hi=KD):
            ps_qk = _grp_tile(lo)
            for kd in range(lo, hi):
                nc.tensor.matmul(
                    ps_qk[:],
                    lhsT=wqk_sb[:, kd, ct * P:(ct + 1) * P],
                    rhs=xT_sb[:, n, kd, :],
                    start=(kd == 0),
                    stop=(kd == KD - 1),
                )
            if hi == KD:
                nc.vector.tensor_scalar_add(
                    qkT_sb[:, ct, n * 512:(n + 1) * 512], ps_qk[:],
                    bqk_sb[:, ct:ct + 1],
                )

        _vpair = [None]

        def emit_v(tt):
            if tt % 2 == 0:
                _vpair[0] = ps_mm.tile([P, 512], F32, tag="mm", name="ps_mm")
            half = (tt % 2) * DL
            ps_v = _vpair[0]
            for kd in range(KD):
                nc.tensor.matmul(
                    ps_v[:, half:half + DL],
                    lhsT=xT_sb[:, tt // 4, kd, (tt % 4) * P:(tt % 4 + 1) * P],
                    rhs=wv_sb[:, kd, :],
                    start=(kd == 0),
                    stop=(kd == KD - 1),
                )
            nc.vector.tensor_copy(v_sb[:, tt, :], ps_v[:, half:half + DL])

        def emit_proj(bi, j, lo=0, hi=KD, g=None):
            if g is None:
                g = g_tiles[bi]
            qs, w = BLOCKS[bi]
            ps_y = _grp_tile(lo)
            for kd in range(lo, hi):
                nc.tensor.matmul(
                    ps_y[:, 0:w],
                    lhsT=wp_sb[:, kd, j * P:(j + 1) * P],
                    rhs=g[:, kd, 0:w],
                    start=(kd == 0),
                    stop=(kd == KD - 1),
                )
            if hi == KD:
                y_sb = yp.tile([P, 512], F32, name="y_sb")
                nc.vector.tensor_scalar_add(y_sb[:, 0:w], ps_y[:, 0:w],
                                            beff_sb[:, j:j + 1])
                nc.sync.dma_start(yT[j * P:(j + 1) * P, qs:qs + w],
                                  y_sb[:, 0:w])

        def emit_scores_pair(ps, bi, kt, pair):
            """Two row-tiled concurrent MMs: heads (2*pair, 2*pair+1).

            ps is [P, 2, 512]; each head's slice sits in its own PSUM bank
            even for narrow blocks (concurrent drains must not share a
            bank+partition range)."""
            qs, qw = BLOCKS[bi]
            for hh in range(2):
                hp = 64 * hh
                nc.tensor.matmul(
                    ps[:, hh, 0:qw],
                    lhsT=qkT_sb[hp:hp + HD, 2 + pair, kt * P:(kt + 1) * P],
                    rhs=qkT_sb[hp:hp + HD, pair, qs:qs + qw],
                    start=True,
                    stop=True,
                )

        def emit_av_pair(kt, e_sb, ps_acc, pair, qw):
            """Two col-tiled concurrent MMs into psum halves 0-63 / 64-127."""
            for hh in range(2):
                h = 2 * pair + hh
                nc.tensor.matmul(
                    ps_acc[64 * hh:64 * hh + HD, 0:qw],
                    lhsT=v_sb[:, kt, h * HD:(h + 1) * HD],
                    rhs=e_sb[:, hh, 0:qw],
                    start=(kt == 0),
                    stop=(kt == KT - 1),
                )

        def emit_rs_quad(kt, rs_tile, e_pair, qw):
            """Rowsum: 4 concurrent 128x1xqw MMs, accumulated over kt."""
            for h in range(HL):
                pair, hh = divmod(h, 2)
                nc.tensor.matmul(
                    rs_tile[32 * h:32 * h + 1, 0:qw],
                    lhsT=ones_sb[:, 0:1],
                    rhs=e_pair[pair][:, hh, 0:qw],
                    start=(kt == 0),
                    stop=(kt == KT - 1),
                    tile_position=(0, 32 * h),
                )

        def make_norm_pair(bi, j, o_sb, r_sb):
            def _norm():
                qs, qw = BLOCKS[bi]
                qmaj, qoff = divmod(qs, QW)
                rb_ps = ps_mm.tile([P, 512], F32, tag="mm", name="ps_mm")
                for hh in range(2):
                    h = 2 * j + hh
                    nc.tensor.matmul(
                        rb_ps[64 * hh:64 * hh + 64, 0:qw],
                        lhsT=ones_sb[32 * h:32 * h + 1, :],
                        rhs=r_sb[32 * h:32 * h + 1, 0:qw],
                        start=True,
                        stop=True,
                        tile_position=(32 * h, 64 * hh),
                    )
                nc.vector.tensor_mul(outT_sb[:, qmaj, j, qoff:qoff + qw],
                                     o_sb[:, 0:qw], rb_ps[:, 0:qw])
            return _norm

        def make_finish(bi, av_tiles, rs_tile):
            """Evacuate AV accumulators; reciprocal frees the rowsum bank."""
            def _fin():
                qw = BLOCKS[bi][1]
                # reciprocal first: it frees the single rowsum PSUM bank the
                # next block's quad is waiting on.  approx_fast is ~5x
                # cheaper than reciprocal() and 18 bits is plenty for the
                # softmax denominator (inputs are positive, well-scaled).
                rf_sb = rp.tile([P, 512], F32, name="rf_sb")
                nc.vector.reciprocal_approx_fast(rf_sb[:, 0:qw],
                                                 rs_tile[:, 0:qw])
                r_sb = rp.tile([P, 512], BF16, name="r_sb")
                nc.vector.tensor_copy(r_sb[:, 0:qw], rf_sb[:, 0:qw])
                o_sb = [rp.tile([P, 512], BF16, name="o0_sb"),
                        rp.tile([P, 512], BF16, name="o1_sb")]
                if bi == NB - 1:   # tail: ScalarE idle; split engines
                    nc.scalar.copy(o_sb[0][:, 0:qw], av_tiles[0][:, 0:qw])
                    nc.scalar.copy(o_sb[1][:, 0:qw], av_tiles[1][:, 0:qw])
                else:
                    nc.vector.tensor_copy(o_sb[0][:, 0:qw], av_tiles[0][:, 0:qw])
                    nc.vector.tensor_copy(o_sb[1][:, 0:qw], av_tiles[1][:, 0:qw])
                return o_sb, r_sb
            return _fin

        def emit_ag(bi, name="g_sb"):
            qs, w = BLOCKS[bi]
            qmaj, qoff = divmod(qs, QW)
            cc_in = dp.tile([2 * P, w], BF16, name="cc_in")
            nc.sync.dma_start(cc_in[:, :].rearrange("(j p) t -> p j t", p=P),
                              outT_sb[:, qmaj, :, qoff:qoff + w])
            cc_out = dp.tile([D, w], BF16, name="cc_out")
            nc.gpsimd.collective_compute(
                "AllGather",
                mybir.AluOpType.bypass,
                replica_groups=GROUPS,
                ins=[cc_in[:, :].opt()],
                outs=[cc_out[:, :].opt()],
            )
            g_sb = gp.tile([P, KD, w], BF16, name=name)
            nc.sync.dma_start(
                g_sb[:], cc_out[:, :].rearrange("(kd p) t -> p kd t", p=P))
            return g_sb

        # ---------------- filler schedule ----------------
        # fillers[(qb, kt, half)] -> thunks run in the exp shadow of that
        # half.  qk/proj groups are split into 4-kd chunks across the two
        # halves of one iteration (ps_mm has a single slot; chunks of a
        # group must stay adjacent among ps_mm users).
        fillers = {}

        def _add(qb, kt, half, fn):
            fillers.setdefault((qb, kt, half), []).append(fn)

        def _add_grp(qb, kt, emitter, *args):
            _add(qb, kt, 0, lambda: emitter(*args, 0, KD // 2))
            _add(qb, kt, 1, lambda: emitter(*args, KD // 2, KD))

        def _add_vpair(qb, kt, tt):
            _add(qb, kt, 0, lambda: emit_v(tt))
            _add(qb, kt, 1, lambda: emit_v(tt + 1))

        # Deadlines (emission order IS dataflow order -- a consumer emitted
        # before its producer reads stale data silently):
        #   v[tt] used by AV at iter tt of block 0 -> v-pair(tt) at kt <= tt-1
        #   k ct2(n)/ct3(n) read by scores(kt=4n) emitted at (block0, 4n-1)
        #     -> both qk groups fully done by kt <= 4n-2
        #   q(n) read one iteration before the first block in token range n.
        #   proj(bi) needs g_tiles[bi]: AllGather takes ~20-25us after its
        #     emission at (bi+1, kt2).
        _add_vpair(0, 0, 2)
        _add_grp(0, 1, emit_qk, 1, 2)
        _add_grp(0, 2, emit_qk, 1, 3)
        _add_vpair(0, 3, 4)
        _add_vpair(0, 4, 6)
        _add_grp(0, 5, emit_qk, 2, 2)
        _add_grp(0, 6, emit_qk, 2, 3)
        _add_vpair(0, 7, 8)
        _add_vpair(0, 8, 10)
        _add_grp(0, 9, emit_qk, 3, 2)
        _add_grp(0, 10, emit_qk, 3, 3)
        _add_vpair(0, 11, 12)
        _add_vpair(0, 12, 14)
        _add_grp(0, 13, emit_qk, 1, 0)
        _add_grp(0, 14, emit_qk, 1, 1)
        _add_grp(1, 3, emit_qk, 2, 0)
        _add_grp(1, 5, emit_qk, 2, 1)
        _add_grp(2, 3, emit_qk, 3, 0)
        _add_grp(2, 5, emit_qk, 3, 1)
        _add_grp(2, 2, emit_proj, 0, 0)
        _add_grp(2, 6, emit_proj, 0, 1)
        _add_grp(3, 4, emit_proj, 1, 0)
        _add_grp(3, 8, emit_proj, 1, 1)
        _add_grp(4, 6, emit_proj, 2, 0)
        _add_grp(4, 10, emit_proj, 2, 1)

        def run_fillers(qb, kt, half):
            for fn in fillers.get((qb, kt, half), ()):
                fn()

        # ---------------- head phase ----------------
        emit_qk(0, 2)      # k pair A, tokens 0-511
        emit_qk(0, 0)      # q pair A, qb0
        sc = [None, None]
        sc[0] = ps_sc.tile([P, 2, 512], F32, tag="sc", name="ps_sc")
        emit_scores_pair(sc[0], 0, 0, 0)
        emit_qk(0, 3)      # k pair B
        emit_qk(0, 1)      # q pair B
        sc[1] = ps_sc.tile([P, 2, 512], F32, tag="sc", name="ps_sc")
        emit_scores_pair(sc[1], 0, 0, 1)
        emit_v(0)
        emit_v(1)

        # ---------------- main loop ----------------
        pend_fin = None       # (bi, fin thunk) awaiting norm/AG scheduling
        fin_out = None
        for bi in range(NB):
            qw = BLOCKS[bi][1]
            av_cur = [None, None]
            rs_cur = None
            e_hist = []
            for kt in range(KT):
                # boundary work of the previous block, before this
                # iteration's halves so the DVE evac/recip precede the new
                # accumulator allocations.
                if pend_fin is not None:
                    pbi = pend_fin[0]
                    if kt == 0:
                        fin_out = pend_fin[1]()
                    elif kt == 1:
                        o_sb, r_sb = fin_out
                        make_norm_pair(pbi, 0, o_sb[0], r_sb)()
                        make_norm_pair(pbi, 1, o_sb[1], r_sb)()
                    elif kt == 2:
                        g_tiles[pbi] = emit_ag(
                            pbi, name="g_sb" if BLOCKS[pbi][1] == QW
                            else "g_half")
                        pend_fin = None
                for pair in range(2):
                    # exp for (bi, kt, pair); frees its sc slot when done.
                    # Both exps are emitted up front: they depend only on
                    # the previous iteration's scores, so ScalarE can run
                    # them back to back.
                    e_sb = ep.tile([P, 2, 512], BF16, name="e_sb")
                    nc.scalar.activation(e_sb[:, :, 0:qw],
                                         sc[pair][:, :, 0:qw], exp_fn,
                                         scale=SCALE)
                    e_hist.append(e_sb)
                # half A: fillers, next scores, AV A(kt), then the lagged
                # AV B(kt-1) in the same 128x64 mode block, then the lagged
                # rowsum quad(kt-1).
                run_fillers(bi, kt, 0)
                if kt + 1 < KT:
                    sc[0] = ps_sc.tile([P, 2, 512], F32, tag="sc",
                                       name="ps_sc")
                    emit_scores_pair(sc[0], bi, kt + 1, 0)
                elif bi + 1 < NB:
                    sc[0] = ps_sc.tile([P, 2, 512], F32, tag="sc",
                                       name="ps_sc")
                    emit_scores_pair(sc[0], bi + 1, 0, 0)
                else:
                    sc[0] = None
                if kt == 0:
                    av_cur[0] = ps_av.tile([P, 512], F32, tag="av",
                                           name="ps_av")
                emit_av_pair(kt, e_hist[-2], av_cur[0], 0, qw)
                if kt > 0:
                    if kt == 1:
                        av_cur[1] = ps_av.tile([P, 512], F32, tag="av",
                                               name="ps_av")
                    emit_av_pair(kt - 1, e_hist[-3], av_cur[1], 1, qw)
                    if kt == 1:
                        rs_cur = ps_rs_pool.tile([P, 512], F32, tag="rs",
                                                 name="ps_rs")
                    emit_rs_quad(kt - 1, rs_cur, e_hist[-4:-2], qw)
                # half B: fillers + next scores only (AV B lags one iter)
                run_fillers(bi, kt, 1)
                if kt + 1 < KT:
                    sc[1] = ps_sc.tile([P, 2, 512], F32, tag="sc",
                                       name="ps_sc")
                    emit_scores_pair(sc[1], bi, kt + 1, 1)
                elif bi + 1 < NB:
                    sc[1] = ps_sc.tile([P, 2, 512], F32, tag="sc",
                                       name="ps_sc")
                    emit_scores_pair(sc[1], bi + 1, 0, 1)
                else:
                    sc[1] = None
                e_hist = e_hist[-4:]

            # flush the lagged pair-B AV and rowsum for kt=15
            emit_av_pair(KT - 1, e_hist[-1], av_cur[1], 1, qw)
            emit_rs_quad(KT - 1, rs_cur, e_hist[-2:], qw)

            pend_fin = (bi, make_finish(bi, av_cur, rs_cur))

        # ---------------- tail: finish last block ----------------
        bi = NB - 1
        o_sb, r_sb = pend_fin[1]()
        make_norm_pair(bi, 0, o_sb[0], r_sb)()
        make_norm_pair(bi, 1, o_sb[1], r_sb)()
        g_tiles[bi] = emit_ag(bi, name="g_half")
        # proj of block 3 overlaps the final AllGather's latency
        emit_proj(NB - 2, 0)
        emit_proj(NB - 2, 1)
        emit_proj(bi, 0)
        emit_proj(bi, 1)


def _build():
    if "nc" in _CACHE:
        return _CACHE["nc"]
    nc = bacc.Bacc(
        "TRN2",
        target_bir_lowering=False,
        debug=False,
        num_devices=NCORES,
    )
    xT = nc.declare_dram_parameter("xT", [QB, P, KD * 512], BF16, isOutput=False)
    wqk = nc.declare_dram_parameter("wqk", [P, KD * 2 * DL], BF16, isOutput=False)
    wv = nc.declare_dram_parameter("wv", [P, KD * DL], BF16, isOutput=False)
    wp = nc.declare_dram_parameter("wp", [P, KD * DL], BF16, isOutput=False)
    bqk = nc.declare_dram_parameter("bqk", [P, 4], F32, isOutput=False)
    beff = nc.declare_dram_parameter("beff", [P, 2], F32, isOutput=False)
    yT = nc.declare_dram_parameter("yT", [DL, S], F32, isOutput=True)

    with tile.TileContext(nc) as tc:
        _emit(nc, tc, xT, wqk, wv, wp, bqk, beff, yT)
    nc.compile()
    _CACHE["nc"] = nc
    return nc


def kernel(x, W_qkv, b_qkv, W_proj, b_proj):
    x = np.asarray(x, np.float32)
    W_qkv = np.asarray(W_qkv, np.float32)
    b_qkv = np.asarray(b_qkv, np.float32)
    W_proj = np.asarray(W_proj, np.float32)
    b_proj = np.asarray(b_proj, np.float32)

    nc = _build()

    b_v = b_qkv[2 * D:3 * D]
    xTt = {}
    for b in range(B):
        xT = np.ascontiguousarray(x[b].T)            # [1024, 2048]
        t = xT.reshape(KD, P, QB, 512).transpose(2, 1, 0, 3)
        xTt[b] = np.ascontiguousarray(t.reshape(QB, P, KD * 512)).astype(NBF16)

    in_maps = []
    for c in range(NCORES):
        b, g = divmod(c, 4)
        cs = DL * g
        wqk_c = np.concatenate(
            [W_qkv[:, cs:cs + DL], W_qkv[:, D + cs:D + cs + DL]], axis=1)
        bqk_c = np.concatenate(
            [b_qkv[cs:cs + DL], b_qkv[D + cs:D + cs + DL]]).reshape(4, P).T
        beff_c = (b_v @ W_proj[:, cs:cs + DL] + b_proj[cs:cs + DL]).reshape(2, P).T
        in_maps.append({
            "xT": xTt[b],
            "wqk": _restripe(wqk_c).astype(NBF16),
            "wv": _restripe(W_qkv[:, 2 * D + cs:2 * D + cs + DL]).astype(NBF16),
            "wp": _restripe(W_proj[:, cs:cs + DL]).astype(NBF16),
            "bqk": np.ascontiguousarray(bqk_c, np.float32),
            "beff": np.ascontiguousarray(beff_c, np.float32),
        })

    trace = bool(int(os.environ.get("TRN_KERNEL_TRACE", "0")))
    res = run_bass_kernel_spmd(nc, in_maps, core_ids=list(range(NCORES)),
                               trace=trace)
    if trace and res.exec_time_ns is not None:
        print(f"HW exec time: {res.exec_time_ns} ns", flush=True)
    _CACHE["last_result"] = res

    out = np.empty((B, S, D), np.float32)
    for c in range(NCORES):
        b, g = divmod(c, 4)
        out[b, :, DL * g:DL * (g + 1)] = res.results[c]["yT"].T
    return out


# revision 3
# speedup vs baseline: 1.0095x; 1.0095x over previous
"""Distributed multi-head attention kernel for one TRN2 chip (8 NeuronCores).

Problem: nn_Attention_13048110645268
  x [2, 2048, 1024] f32 ->  attention(16 heads, d=64) -> out [2, 2048, 1024] f32

Sharding (Megatron-style batch x head-group):
  core c in [0,8): batch b = c//4, head group g = c%4 (heads 4g..4g+3).

v3 design: ScalarE-exp-clocked software pipeline with PE tile concurrency.
  - Heads processed as two pairs A=(0,1), B=(2,3); qk features of the even
    head on partitions 0-63, odd head on 64-127.
  - scores pair: two 64x128x512 MMs, lhsT base partitions 0/64 -> row-tiled,
    run concurrently on the PE.
  - AV pair: two 128x64x512 MMs into psum halves 0-63/64-127 -> col-tiled,
    concurrent.
  - exp: one [128,1024] ScalarE activation per pair per key tile; ScalarE
    is the pipeline clock (~2.5us/iter at the P0 clock).
  - rowsum: one 4-way-concurrent 128x1x512 quad per iteration accumulating
    into a dedicated PSUM bank (freed early by the boundary reciprocal).
  - PSUM banks: scores 2x[128,1024] (4) + AV accums 2x[128,512] (2) +
    rowsum (1) + filler scratch (1) = 8.
  - qk/v/proj run as deadline-scheduled fillers inside the exp shadow,
    with qk/proj groups split into 4-kd chunks across the two halves of
    one iteration.
  - 5 query blocks (512,512,512,256,256): the two final 256-query
    blocks let the second-to-last AllGather hide under attention and
    leave only a small AG exposed in the tail.
"""

import os
import sys

import numpy as np

sys.path.insert(0, "/opt/trn_rl_repo")

import ml_dtypes  # noqa: E402

import concourse.bass as bass  # noqa: E402
import concourse.mybir as mybir  # noqa: E402
import concourse.tile as tile  # noqa: E402
from concourse import bacc  # noqa: E402
from concourse.bass_utils import run_bass_kernel_spmd  # noqa: E402

BF16 = mybir.dt.bfloat16
F32 = mybir.dt.float32
NBF16 = ml_dtypes.bfloat16

B, S, D = 2, 2048, 1024
H, HD = 16, 64
NCORES = 8
GROUPS = [[0, 1, 2, 3], [4, 5, 6, 7]]
HL = 4          # heads per core
DL = HL * HD    # 256 feature dims per core
P = 128
KT = S // P     # 16 key tiles
QB = 4          # query blocks
QW = S // QB    # 512 queries per block
KD = D // P     # 8 contraction tiles over model dim
SCALE = HD ** -0.5
# (global query start, width) per processing block; last 512 split in two so
# the second-to-last AllGather hides under the final block's attention.
BLOCKS = [(0, 512), (512, 512), (1024, 512), (1536, 256), (1792, 256)]
NB = len(BLOCKS)

_CACHE = {}


def _restripe(w):
    """[KD*128, C] -> [128, KD*C] with row p holding all kd-subtiles."""
    kd = w.shape[0] // P
    return np.ascontiguousarray(
        w.reshape(kd, P, w.shape[1]).transpose(1, 0, 2).reshape(P, -1))


def _emit(nc: bass.Bass, tc: tile.TileContext, xT, wqk, wv, wp, bqk, beff, yT):
    exp_fn = mybir.ActivationFunctionType.Exp

    with (
        tc.tile_pool(name="main", bufs=1) as mp,
        tc.tile_pool(name="ep", bufs=4) as ep,
        tc.tile_pool(name="esp", bufs=4) as esp,
        tc.tile_pool(name="gp", bufs=2) as gp,
        tc.tile_pool(name="yp", bufs=2) as yp,
        tc.tile_pool(name="rp", bufs=4) as rp,
        tc.tile_pool(name="ps_sc", bufs=2, space="PSUM") as ps_sc,
        tc.tile_pool(name="ps_av", bufs=2, space="PSUM") as ps_av,
        tc.tile_pool(name="ps_mm", bufs=2, space="PSUM") as ps_mm,
        tc.tile_pool(name="dram", bufs=2, space="DRAM") as dp,
    ):
        # ---------------- input DMA (ordered by first use) ----------------
        xT_sb = mp.tile([P, QB, KD, 512], BF16)   # x^T [d-part, n, d-tile, tok]
        nc.sync.dma_start(xT_sb[:, 0],
                          xT[0, :, :].rearrange("p (kd u) -> p kd u", kd=KD))
        wqk_sb = mp.tile([P, KD, 2 * DL], BF16)
        nc.sync.dma_start(wqk_sb[:],
                          wqk[:, :].rearrange("p (kd c) -> p kd c", kd=KD))
        bqk_sb = mp.tile([P, 4], F32)
        nc.sync.dma_start(bqk_sb[:], bqk[:, :])
        wv_sb = mp.tile([P, KD, DL], BF16)
        nc.sync.dma_start(wv_sb[:],
                          wv[:, :].rearrange("p (kd c) -> p kd c", kd=KD))
        for n in range(1, QB):
            nc.sync.dma_start(xT_sb[:, n],
                              xT[n, :, :].rearrange("p (kd u) -> p kd u", kd=KD))
        wp_sb = mp.tile([P, KD, DL], BF16)
        nc.sync.dma_start(wp_sb[:],
                          wp[:, :].rearrange("p (kd c) -> p kd c", kd=KD))
        beff_sb = mp.tile([P, 2], F32)
        nc.sync.dma_start(beff_sb[:], beff[:, :])
        ones_sb = mp.tile([P, 64], BF16)
        nc.vector.memset(ones_sb[:], 1.0)
        warm_cc_in = dp.tile([8, 64], BF16, name="warm_cc_in")
        nc.sync.dma_start(warm_cc_in[:, :], xT[0, 0:8, 0:64])
        warm_cc_out = dp.tile([32, 64], BF16, name="warm_cc_out")
        nc.gpsimd.collective_compute(
            "AllGather",
            mybir.AluOpType.bypass,
            replica_groups=GROUPS,
            ins=[warm_cc_in[:, :].opt()],
            outs=[warm_cc_out[:, :].opt()],
        )

        # ---------------- persistent SBUF ----------------
        qkT_sb = mp.tile([P, 4, S], BF16)     # ct 0,1 = q pair A/B; 2,3 = k
        v_sb = mp.tile([P, KT, DL], BF16)
        outT_sb = mp.tile([P, QB, 2, 512], BF16)
        g_tiles = [None] * NB

        # PE warm-up: dummy matmuls with no input deps run while input DMAs
        # land, lifting the HAM clock gate before real work.
        warm_sb = mp.tile([P, 512], BF16)
        nc.vector.memset(warm_sb[:], 1.0)
        for w in range(10):
            ps_warm = ps_mm.tile([P, 512], F32, tag="mm", name="ps_mm")
            nc.tensor.matmul(ps_warm[:], lhsT=warm_sb[:, 0:P], rhs=warm_sb[:],
                             start=True, stop=True)

        # ---------------- building blocks ----------------
        _grp = [None]   # psum tile held across the chunks of one mm group

        def _grp_tile(lo):
            if lo == 0:
                _grp[0] = ps_mm.tile([P, 512], F32, tag="mm", name="ps_mm")
            return _grp[0]

        def emit_qk(n, ct, lo=0, hi=KD):
            ps_qk = _grp_tile(lo)
            for kd in range(lo, hi):
                nc.tensor.matmul(
                    ps_qk[:],
                    lhsT=wqk_sb[:, kd, ct * P:(ct + 1) * P],
                    rhs=xT_sb[:, n, kd, :],
                    start=(kd == 0),
                    stop=(kd == KD - 1),
                )
            if hi == KD:
                nc.vector.tensor_scalar_add(
                    qkT_sb[:, ct, n * 512:(n + 1) * 512], ps_qk[:],
                    bqk_sb[:, ct:ct + 1],
                )

        _vpair = [None]

        def emit_v(tt):
            if tt % 2 == 0:
                _vpair[0] = ps_mm.tile([P, 512], F32, tag="mm", name="ps_mm")
            half = (tt % 2) * DL
            ps_v = _vpair[0]
            for kd in range(KD):
                nc.tensor.matmul(
                    ps_v[:, half:half + DL],
                    lhsT=xT_sb[:, tt // 4, kd, (tt % 4) * P:(tt % 4 + 1) * P],
                    rhs=wv_sb[:, kd, :],
                    start=(kd == 0),
                    stop=(kd == KD - 1),
                )
            nc.vector.tensor_copy(v_sb[:, tt, :], ps_v[:, half:half + DL])

        def emit_proj(bi, j, lo=0, hi=KD, g=None):
            if g is None:
                g = g_tiles[bi]
            qs, w = BLOCKS[bi]
            ps_y = _grp_tile(lo)
            for kd in range(lo, hi):
                nc.tensor.matmul(
                    ps_y[:, 0:w],
                    lhsT=wp_sb[:, kd, j * P:(j + 1) * P],
                    rhs=g[:, kd, 0:w],
                    start=(kd == 0),
                    stop=(kd == KD - 1),
                )
            if hi == KD:
                y_sb = yp.tile([P, 512], F32, name="y_sb")
                nc.vector.tensor_scalar_add(y_sb[:, 0:w], ps_y[:, 0:w],
                                            beff_sb[:, j:j + 1])
                nc.sync.dma_start(yT[j * P:(j + 1) * P, qs:qs + w],
                                  y_sb[:, 0:w])

        def emit_scores_pair(ps, bi, kt, pair):
            """Two row-tiled concurrent MMs: heads (2*pair, 2*pair+1).

            ps is [P, 2, 512]; each head's slice sits in its own PSUM bank
            even for narrow blocks (concurrent drains must not share a
            bank+partition range)."""
            qs, qw = BLOCKS[bi]
            for hh in range(2):
                hp = 64 * hh
                nc.tensor.matmul(
                    ps[:, hh, 0:qw],
                    lhsT=qkT_sb[hp:hp + HD, 2 + pair, kt * P:(kt + 1) * P],
                    rhs=qkT_sb[hp:hp + HD, pair, qs:qs + qw],
                    start=True,
                    stop=True,
                )

        def emit_av_pair(kt, e_sb, ps_acc, pair, qw):
            """Two col-tiled concurrent MMs into psum halves 0-63 / 64-127."""
            for hh in range(2):
                h = 2 * pair + hh
                nc.tensor.matmul(
                    ps_acc[64 * hh:64 * hh + HD, 0:qw],
                    lhsT=v_sb[:, kt, h * HD:(h + 1) * HD],
                    rhs=e_sb[:, hh, 0:qw],
                    start=(kt == 0),
                    stop=(kt == KT - 1),
                )

        def emit_rs_reduce(ps_rs, es_tiles, qw):
            """Rowsums from the DVE-accumulated E sums: 4 concurrent MMs."""
            for h in range(HL):
                pair, hh = divmod(h, 2)
                nc.tensor.matmul(
                    ps_rs[32 * h:32 * h + 1, 0:qw],
                    lhsT=ones_sb[:, 0:1],
                    rhs=es_tiles[pair][:, hh, 0:qw],
                    start=True,
                    stop=True,
                    tile_position=(0, 32 * h),
                )

        def make_norm_pair(bi, j, o_sb, r_sb):
            def _norm():
                qs, qw = BLOCKS[bi]
                qmaj, qoff = divmod(qs, QW)
                rb_ps = ps_mm.tile([P, 512], F32, tag="mm", name="ps_mm")
                for hh in range(2):
                    h = 2 * j + hh
                    nc.tensor.matmul(
                        rb_ps[64 * hh:64 * hh + 64, 0:qw],
                        lhsT=ones_sb[32 * h:32 * h + 1, :],
                        rhs=r_sb[32 * h:32 * h + 1, 0:qw],
                        start=True,
                        stop=True,
                        tile_position=(32 * h, 64 * hh),
                    )
                nc.vector.tensor_mul(outT_sb[:, qmaj, j, qoff:qoff + qw],
                                     o_sb[:, 0:qw], rb_ps[:, 0:qw])
            return _norm

        def make_finish(bi, av_tiles, es_tiles):
            """Rowsum-reduce + reciprocal + AV evacuation."""
            def _fin():
                qw = BLOCKS[bi][1]
                ps_rs = ps_mm.tile([P, 512], F32, tag="mm", name="ps_mm")
                emit_rs_reduce(ps_rs, es_tiles, qw)
                rf_sb = rp.tile([P, 512], F32, name="rf_sb")
                nc.vector.reciprocal_approx_fast(rf_sb[:, 0:qw],
                                                 ps_rs[:, 0:qw])
                r_sb = rp.tile([P, 512], BF16, name="r_sb")
                nc.vector.tensor_copy(r_sb[:, 0:qw], rf_sb[:, 0:qw])
                o_sb = [rp.tile([P, 512], BF16, name="o0_sb"),
                        rp.tile([P, 512], BF16, name="o1_sb")]
                if bi == NB - 1:   # tail: ScalarE idle; split engines
                    nc.scalar.copy(o_sb[0][:, 0:qw], av_tiles[0][:, 0:qw])
                    nc.scalar.copy(o_sb[1][:, 0:qw], av_tiles[1][:, 0:qw])
                else:
                    nc.vector.tensor_copy(o_sb[0][:, 0:qw], av_tiles[0][:, 0:qw])
                    nc.vector.tensor_copy(o_sb[1][:, 0:qw], av_tiles[1][:, 0:qw])
                return o_sb, r_sb
            return _fin

        def emit_ag(bi, name="g_sb"):
            qs, w = BLOCKS[bi]
            qmaj, qoff = divmod(qs, QW)
            cc_in = dp.tile([2 * P, w], BF16, name="cc_in")
            nc.sync.dma_start(cc_in[:, :].rearrange("(j p) t -> p j t", p=P),
                              outT_sb[:, qmaj, :, qoff:qoff + w])
            cc_out = dp.tile([D, w], BF16, name="cc_out")
            nc.gpsimd.collective_compute(
                "AllGather",
                mybir.AluOpType.bypass,
                replica_groups=GROUPS,
                ins=[cc_in[:, :].opt()],
                outs=[cc_out[:, :].opt()],
            )
            g_sb = gp.tile([P, KD, w], BF16, name=name)
            nc.sync.dma_start(
                g_sb[:], cc_out[:, :].rearrange("(kd p) t -> p kd t", p=P))
            return g_sb

        # ---------------- filler schedule ----------------
        # fillers[(qb, kt, half)] -> thunks run in the exp shadow of that
        # half.  qk/proj groups are split into 4-kd chunks across the two
        # halves of one iteration (ps_mm has a single slot; chunks of a
        # group must stay adjacent among ps_mm users).
        fillers = {}

        def _add(qb, kt, half, fn):
            fillers.setdefault((qb, kt, half), []).append(fn)

        def _add_grp(qb, kt, emitter, *args):
            _add(qb, kt, 0, lambda: emitter(*args, 0, KD // 2))
            _add(qb, kt, 1, lambda: emitter(*args, KD // 2, KD))

        def _add_vpair(qb, kt, tt):
            _add(qb, kt, 0, lambda: emit_v(tt))
            _add(qb, kt, 1, lambda: emit_v(tt + 1))

        # Deadlines (emission order IS dataflow order -- a consumer emitted
        # before its producer reads stale data silently):
        #   v[tt] used by AV at iter tt of block 0 -> v-pair(tt) at kt <= tt-1
        #   k ct2(n)/ct3(n) read by scores(kt=4n) emitted at (block0, 4n-1)
        #     -> both qk groups fully done by kt <= 4n-2
        #   q(n) read one iteration before the first block in token range n.
        #   proj(bi) needs g_tiles[bi]: AllGather takes ~20-25us after its
        #     emission at (bi+1, kt2).
        _add_vpair(0, 0, 2)
        _add_grp(0, 1, emit_qk, 1, 2)
        _add_grp(0, 2, emit_qk, 1, 3)
        _add_vpair(0, 3, 4)
        _add_vpair(0, 4, 6)
        _add_grp(0, 5, emit_qk, 2, 2)
        _add_grp(0, 6, emit_qk, 2, 3)
        _add_vpair(0, 7, 8)
        _add_vpair(0, 8, 10)
        _add_grp(0, 9, emit_qk, 3, 2)
        _add_grp(0, 10, emit_qk, 3, 3)
        _add_vpair(0, 11, 12)
        _add_vpair(0, 12, 14)
        _add_grp(0, 13, emit_qk, 1, 0)
        _add_grp(0, 14, emit_qk, 1, 1)
        _add_grp(1, 3, emit_qk, 2, 0)
        _add_grp(1, 5, emit_qk, 2, 1)
        _add_grp(2, 3, emit_qk, 3, 0)
        _add_grp(2, 5, emit_qk, 3, 1)
        _add_grp(2, 2, emit_proj, 0, 0)
        _add_grp(2, 6, emit_proj, 0, 1)
        _add_grp(3, 4, emit_proj, 1, 0)
        _add_grp(3, 8, emit_proj, 1, 1)
        _add_grp(4, 6, emit_proj, 2, 0)
        _add_grp(4, 10, emit_proj, 2, 1)

        def run_fillers(qb, kt, half):
            for fn in fillers.get((qb, kt, half), ()):
                fn()

        # ---------------- head phase ----------------
        emit_qk(0, 2)      # k pair A, tokens 0-511
        emit_qk(0, 0)      # q pair A, qb0
        sc = [None, None]
        sc[0] = ps_sc.tile([P, 2, 512], F32, tag="sc", name="ps_sc")
        emit_scores_pair(sc[0], 0, 0, 0)
        emit_qk(0, 3)      # k pair B
        emit_qk(0, 1)      # q pair B
        sc[1] = ps_sc.tile([P, 2, 512], F32, tag="sc", name="ps_sc")
        emit_scores_pair(sc[1], 0, 0, 1)
        emit_v(0)
        emit_v(1)

        # ---------------- main loop ----------------
        pend_fin = None       # (bi, fin thunk) awaiting norm/AG scheduling
        fin_out = None
        for bi in range(NB):
            qw = BLOCKS[bi][1]
            av_cur = [None, None]
            es_cur = [None, None]
            e_hist = []
            for kt in range(KT):
                # boundary work of the previous block, before this
                # iteration's halves so the DVE evac/recip precede the new
                # accumulator allocations.
                if pend_fin is not None:
                    pbi = pend_fin[0]
                    if kt == 0:
                        fin_out = pend_fin[1]()
                    elif kt == 1:
                        o_sb, r_sb = fin_out
                        make_norm_pair(pbi, 0, o_sb[0], r_sb)()
                        make_norm_pair(pbi, 1, o_sb[1], r_sb)()
                    elif kt == 2:
                        g_tiles[pbi] = emit_ag(
                            pbi, name="g_sb" if BLOCKS[pbi][1] == QW
                            else "g_half")
                        pend_fin = None
                for pair in range(2):
                    # exp for (bi, kt, pair); frees its sc slot when done.
                    # Both exps are emitted up front: they depend only on
                    # the previous iteration's scores, so ScalarE can run
                    # them back to back.
                    e_sb = ep.tile([P, 2, 512], BF16, name="e_sb")
                    nc.scalar.activation(e_sb[:, :, 0:qw],
                                         sc[pair][:, :, 0:qw], exp_fn,
                                         scale=SCALE)
                    if kt == 0:
                        es_cur[pair] = esp.tile([P, 2, 512], BF16,
                                                name="esum")
                        nc.vector.tensor_copy(es_cur[pair][:, :, 0:qw],
                                              e_sb[:, :, 0:qw])
                    else:
                        nc.vector.tensor_add(es_cur[pair][:, :, 0:qw],
                                             es_cur[pair][:, :, 0:qw],
                                             e_sb[:, :, 0:qw])
                    e_hist.append(e_sb)
                # half A: fillers, next scores, AV A(kt), then the lagged
                # AV B(kt-1) in the same 128x64 mode block, then the lagged
                # rowsum quad(kt-1).
                run_fillers(bi, kt, 0)
                if kt + 1 < KT:
                    sc[0] = ps_sc.tile([P, 2, 512], F32, tag="sc",
                                       name="ps_sc")
                    emit_scores_pair(sc[0], bi, kt + 1, 0)
                elif bi + 1 < NB:
                    sc[0] = ps_sc.tile([P, 2, 512], F32, tag="sc",
                                       name="ps_sc")
                    emit_scores_pair(sc[0], bi + 1, 0, 0)
                else:
                    sc[0] = None
                if kt == 0:
                    av_cur[0] = ps_av.tile([P, 512], F32, tag="av",
                                           name="ps_av")
                emit_av_pair(kt, e_hist[-2], av_cur[0], 0, qw)
                if kt > 0:
                    if kt == 1:
                        av_cur[1] = ps_av.tile([P, 512], F32, tag="av",
                                               name="ps_av")
                    emit_av_pair(kt - 1, e_hist[-3], av_cur[1], 1, qw)
                # half B: fillers + next scores only (AV B lags one iter)
                run_fillers(bi, kt, 1)
                if kt + 1 < KT:
                    sc[1] = ps_sc.tile([P, 2, 512], F32, tag="sc",
                                       name="ps_sc")
                    emit_scores_pair(sc[1], bi, kt + 1, 1)
                elif bi + 1 < NB:
                    sc[1] = ps_sc.tile([P, 2, 512], F32, tag="sc",
                                       name="ps_sc")
                    emit_scores_pair(sc[1], bi + 1, 0, 1)
                else:
                    sc[1] = None
                e_hist = e_hist[-4:]

            # flush the lagged pair-B AV for kt=15
            emit_av_pair(KT - 1, e_hist[-1], av_cur[1], 1, qw)

            pend_fin = (bi, make_finish(bi, av_cur, es_cur))

        # ---------------- tail: finish last block ----------------
        bi = NB - 1
        o_sb, r_sb = pend_fin[1]()
        make_norm_pair(bi, 0, o_sb[0], r_sb)()
        make_norm_pair(bi, 1, o_sb[1], r_sb)()
        g_tiles[bi] = emit_ag(bi, name="g_half")
        # proj of block 3 overlaps the final AllGather's latency
        emit_proj(NB - 2, 0)
        emit_proj(NB - 2, 1)
        emit_proj(bi, 0)
        emit_proj(bi, 1)


def _build():
    if "nc" in _CACHE:
        return _CACHE["nc"]
    nc = bacc.Bacc(
        "TRN2",
        target_bir_lowering=False,
        debug=False,
        num_devices=NCORES,
    )
    xT = nc.declare_dram_parameter("xT", [QB, P, KD * 512], BF16, isOutput=False)
    wqk = nc.declare_dram_parameter("wqk", [P, KD * 2 * DL], BF16, isOutput=False)
    wv = nc.declare_dram_parameter("wv", [P, KD * DL], BF16, isOutput=False)
    wp = nc.declare_dram_parameter("wp", [P, KD * DL], BF16, isOutput=False)
    bqk = nc.declare_dram_parameter("bqk", [P, 4], F32, isOutput=False)
    beff = nc.declare_dram_parameter("beff", [P, 2], F32, isOutput=False)
    yT = nc.declare_dram_parameter("yT", [DL, S], F32, isOutput=True)

    with tile.TileContext(nc) as tc:
        _emit(nc, tc, xT, wqk, wv, wp, bqk, beff, yT)
    nc.compile()
    _CACHE["nc"] = nc
    return nc


def kernel(x, W_qkv, b_qkv, W_proj, b_proj):
    x = np.asarray(x, np.float32)
    W_qkv = np.asarray(W_qkv, np.float32)
    b_qkv = np.asarray(b_qkv, np.float32)
    W_proj = np.asarray(W_proj, np.float32)
    b_proj = np.asarray(b_proj, np.float32)

    nc = _build()

    b_v = b_qkv[2 * D:3 * D]
    xTt = {}
    for b in range(B):
        xT = np.ascontiguousarray(x[b].T)            # [1024, 2048]
        t = xT.reshape(KD, P, QB, 512).transpose(2, 1, 0, 3)
        xTt[b] = np.ascontiguousarray(t.reshape(QB, P, KD * 512)).astype(NBF16)

    in_maps = []
    for c in range(NCORES):
        b, g = divmod(c, 4)
        cs = DL * g
        wqk_c = np.concatenate(
            [W_qkv[:, cs:cs + DL], W_qkv[:, D + cs:D + cs + DL]], axis=1)
        bqk_c = np.concatenate(
            [b_qkv[cs:cs + DL], b_qkv[D + cs:D + cs + DL]]).reshape(4, P).T
        beff_c = (b_v @ W_proj[:, cs:cs + DL] + b_proj[cs:cs + DL]).reshape(2, P).T
        in_maps.append({
            "xT": xTt[b],
            "wqk": _restripe(wqk_c).astype(NBF16),
            "wv": _restripe(W_qkv[:, 2 * D + cs:2 * D + cs + DL]).astype(NBF16),
            "wp": _restripe(W_proj[:, cs:cs + DL]).astype(NBF16),
            "bqk": np.ascontiguousarray(bqk_c, np.float32),
            "beff": np.ascontiguousarray(beff_c, np.float32),
        })

    trace = bool(int(os.environ.get("TRN_KERNEL_TRACE", "0")))
    res = run_bass_kernel_spmd(nc, in_maps, core_ids=list(range(NCORES)),
                               trace=trace)
    if trace and res.exec_time_ns is not None:
        print(f"HW exec time: {res.exec_time_ns} ns", flush=True)
    _CACHE["last_result"] = res

    out = np.empty((B, S, D), np.float32)
    for c in range(NCORES):
        b, g = divmod(c, 4)
        out[b, :, DL * g:DL * (g + 1)] = res.results[c]["yT"].T
    return out
